# revision 6
# baseline (speedup 1.0000x reference)
"""Multi-head attention (B=2, S=2048, D=1024, H=16) on 8 NeuronCores.

Sharding: core c -> batch b = c//4, head group g = c%4 (4 heads each).
Each core computes q/k/v projections for its head group (bias folded in via an
augmented ones-row on x and a bias-row on W), full softmax attention for its 4
heads, and a partial output projection out_c = attn_out_c @ Wo[rows_c].  The
host sums the 4 partials per batch and adds bo.

Kernel layout (per core, all fp32):
  - xT [1152, 2048] = [x.T; ones; zeros]   (K padded 1024+1 -> 9*128)
  - qT/kT [256, 2048] computed directly in transposed layout (heads on
    partitions, 2 tiles of 128 = 2 head-pairs), v in natural layout with an
    extra ones column per head (v_aug) so the attention-output matmul also
    accumulates the softmax denominator as row 64.
  - scoresT[sk, sq] = k @ qT per 128-row k-block, row-tiled 2 heads per pass
    (K=64 each at row groups 0/64); exp on ScalarE straight out of PSUM
    (no max subtraction: scores ~ N(0,1), max << 88).
  - outT[65, sq] accumulates over k-blocks in PSUM; row 64 is the denominator.
    Normalization: reciprocal of row 64, partition-broadcast via DMA, fused
    into the PSUM->SBUF evacuation multiply.
  - projection: out[s, n] = sum_h outTs_h[dv, s].T @ Wo_h[dv, n], K=64 chunks.
"""

import numpy as np

S = 2048
D = 1024
H = 16
DEPTH = 64
NCORES = 8
GH = 4              # heads per core
GD = GH * DEPTH     # 256 output dims per core
KC = 9              # contraction chunks of 128 (1024 data + 1 bias + pad)
KAUG = KC * 128     # 1152

_state = {}


def _build():
    import concourse.mybir as mybir
    import concourse.tile as tile
    from concourse import bacc
    from concourse.bass import ts

    fp32 = mybir.dt.float32
    Exp = mybir.ActivationFunctionType.Exp

    nc = bacc.Bacc("TRN2", target_bir_lowering=False, debug=False)
    xT = nc.dram_tensor("xT", [KAUG, S], fp32, kind="ExternalInput")
    wq = nc.dram_tensor("wq", [KAUG, GD], fp32, kind="ExternalInput")
    wk = nc.dram_tensor("wk", [KAUG, GD], fp32, kind="ExternalInput")
    wv = nc.dram_tensor("wv", [KAUG, GD], fp32, kind="ExternalInput")
    wo = nc.dram_tensor("wo", [GD, D], fp32, kind="ExternalInput")
    out = nc.dram_tensor("out", [S, D], fp32, kind="ExternalOutput")

    with tile.TileContext(nc) as tc:
        with tc.tile_pool(name="singles", bufs=1) as singles:
            qT = singles.tile([128, 2, S], fp32)       # [dout%128, pair, sq]
            kT = singles.tile([128, 2, S], fp32)
            v_sb = singles.tile([128, 16, GH, DEPTH + 1], fp32)  # v_aug
            # rows 0-63: unnormalized attn out^T (normalized in place later);
            # row 64: reciprocal softmax denominators
            outTs = singles.tile([65, GH, S], fp32)
            nc.vector.memset(v_sb[:, :, :, DEPTH : DEPTH + 1], 1.0)

            # ---------- phase 1: QKV projections ----------
            with (
                tc.tile_pool(name="wpool", bufs=1) as wpool,
                tc.tile_pool(name="xpool", bufs=2) as xpool,
                tc.tile_pool(name="ps1", bufs=3, space="PSUM") as ps1,
            ):
                wq_sb = wpool.tile([128, KC, GD], fp32)
                wk_sb = wpool.tile([128, KC, GD], fp32)
                wv_sb = wpool.tile([128, KC, GD], fp32)
                nc.sync.dma_start(wq_sb[:], wq[:].rearrange("(c p) d -> p c d", p=128))
                nc.sync.dma_start(wk_sb[:], wk[:].rearrange("(c p) d -> p c d", p=128))
                nc.sync.dma_start(wv_sb[:], wv[:].rearrange("(c p) d -> p c d", p=128))
                xT_view = xT[:].rearrange("(c p) s -> p c s", p=128)

                for sc in range(4):  # s-chunks of 512
                    xc = xpool.tile([128, KC, 512], fp32, tag="xc")
                    nc.sync.dma_start(xc[:], xT_view[:, :, ts(sc, 512)])
                    for hp in range(2):
                        for w_sb, dst in ((wq_sb, qT), (wk_sb, kT)):
                            ps = ps1.tile([128, 512], fp32, tag="pq")
                            for kc in range(KC):
                                nc.tensor.matmul(
                                    ps[:],
                                    w_sb[:, kc, ts(hp, 128)],
                                    xc[:, kc, :],
                                    start=(kc == 0),
                                    stop=(kc == KC - 1),
                                )
                            nc.vector.tensor_copy(dst[:, hp, ts(sc, 512)], ps[:])
                    for mm in range(4):  # s-blocks of 128 inside the chunk
                        ps = ps1.tile([128, GD], fp32, tag="pv")
                        for kc in range(KC):
                            nc.tensor.matmul(
                                ps[:],
                                xc[:, kc, ts(mm, 128)],
                                wv_sb[:, kc, :],
                                start=(kc == 0),
                                stop=(kc == KC - 1),
                            )
                        nc.vector.tensor_copy(
                            v_sb[:, sc * 4 + mm, :, 0:DEPTH],
                            ps[:].rearrange("p (h d) -> p h d", h=GH),
                        )

            # ---------- phase 2: attention ----------
            recip_dram = nc.dram_tensor("recip_scratch", [GH * S], fp32)
            with (
                tc.tile_pool(name="expp", bufs=3) as expp,
                tc.tile_pool(name="rbp", bufs=1) as rbp,
                tc.tile_pool(name="pss", bufs=2, space="PSUM") as pss,
                tc.tile_pool(name="pso", bufs=4, space="PSUM") as pso,
            ):
                for hp in range(2):
                    for sqc in range(4):
                        oAB = [
                            pso.tile([65, 512], fp32, tag="o", name=f"o{a}")
                            for a in range(2)
                        ]
                        for kb in range(16):
                            sps = pss.tile([128, 2, 512], fp32, tag="s")
                            # scoresT = k @ qT, two heads row-tiled (K=64)
                            for a in range(2):
                                nc.tensor.matmul(
                                    sps[:, a, :],
                                    kT[a * 64 : (a + 1) * 64, hp, ts(kb, 128)],
                                    qT[a * 64 : (a + 1) * 64, hp, ts(sqc, 512)],
                                    start=True,
                                    stop=True,
                                )
                            ex = expp.tile([128, 2, 512], fp32, tag="e")
                            nc.scalar.activation(ex[:], sps[:], Exp, scale=0.125)
                            # outT[65, sq] += v_aug.T @ expT  (row 64 = denom)
                            for a in range(2):
                                nc.tensor.matmul(
                                    oAB[a][:],
                                    v_sb[:, kb, 2 * hp + a, :],
                                    ex[:, a, :],
                                    start=(kb == 0),
                                    stop=(kb == 15),
                                )
                        for a in range(2):
                            h = 2 * hp + a
                            nc.vector.reciprocal(
                                outTs[64:65, h, ts(sqc, 512)], oAB[a][64:65, :]
                            )
                            nc.vector.tensor_copy(
                                outTs[0:64, h, ts(sqc, 512)], oAB[a][0:64, :]
                            )

                # broadcast reciprocal denominators across the 64 dv partitions
                # (SBUF partition-broadcast needs a DRAM bounce)
                rbAll = rbp.tile([64, GH, S], fp32)
                recip_view = recip_dram[:].rearrange("(p h s) -> p h s", p=1, h=GH)
                nc.sync.dma_start(recip_view, outTs[64:65, :, :])
                nc.sync.dma_start(rbAll[:], recip_view.to_broadcast([64, GH, S]))
                # normalize in place, per s-block so projection can pipeline
                for m in range(16):
                    nc.vector.tensor_mul(
                        outTs[0:64, :, ts(m, 128)],
                        outTs[0:64, :, ts(m, 128)],
                        rbAll[:, :, ts(m, 128)],
                    )

            # ---------- phase 3: output projection ----------
            with (
                tc.tile_pool(name="wop", bufs=1) as wop,
                tc.tile_pool(name="outp", bufs=3) as outp,
                tc.tile_pool(name="ps3", bufs=4, space="PSUM") as ps3,
            ):
                wo_sb = wop.tile([64, GH, D], fp32)
                nc.sync.dma_start(wo_sb[:], wo[:].rearrange("(h p) n -> p h n", p=64))
                for m in range(16):  # s-blocks of 128
                    ot = outp.tile([128, D], fp32, tag="ot")
                    for nn in range(2):  # n-chunks of 512
                        ps = ps3.tile([128, 512], fp32, tag="po")
                        for h in range(GH):
                            nc.tensor.matmul(
                                ps[:],
                                outTs[0:64, h, ts(m, 128)],
                                wo_sb[:, h, ts(nn, 512)],
                                start=(h == 0),
                                stop=(h == GH - 1),
                            )
                        nc.vector.tensor_copy(ot[:, ts(nn, 512)], ps[:])
                    nc.sync.dma_start(out[m * 128 : (m + 1) * 128, :], ot[:])

    nc.compile()
    return nc


def _get_nc():
    if "nc" not in _state:
        _state["nc"] = _build()
    return _state["nc"]


def _prep_core_inputs(inputs, Wq, bq, Wk, bk, Wv, bv, Wo, bo):
    """Build the 8 per-core input dicts (host-side shard + transpose + bias fold)."""
    in_maps = []
    for c in range(NCORES):
        b, g = divmod(c, 4)
        cols = slice(g * GD, (g + 1) * GD)
        xTa = np.zeros((KAUG, S), np.float32)
        xTa[:D] = inputs[b].T
        xTa[D] = 1.0
        m = {"xT": xTa}
        for name, W, bias in (("wq", Wq, bq), ("wk", Wk, bk), ("wv", Wv, bv)):
            Wa = np.zeros((KAUG, GD), np.float32)
            Wa[:D] = W[:, cols]
            Wa[D] = bias[cols]
            m[name] = Wa
        m["wo"] = np.ascontiguousarray(Wo[cols, :], dtype=np.float32)
        in_maps.append(m)
    return in_maps


def run(inputs, Wq, bq, Wk, bk, Wv, bv, Wo, bo, trace=False):
    from concourse.bass_utils import run_bass_kernel_spmd

    nc = _get_nc()
    in_maps = _prep_core_inputs(inputs, Wq, bq, Wk, bk, Wv, bv, Wo, bo)
    res = run_bass_kernel_spmd(
        nc, in_maps, core_ids=list(range(NCORES)), trace=trace
    )
    out = np.zeros((2, S, D), np.float32)
    for c in range(NCORES):
        out[c // 4] += res.results[c]["out"]
    out += np.asarray(bo, np.float32)
    return out, res


def kernel(inputs, Wq, bq, Wk, bk, Wv, bv, Wo, bo):
    out, _ = run(
        np.asarray(inputs, np.float32),
        np.asarray(Wq, np.float32), np.asarray(bq, np.float32),
        np.asarray(Wk, np.float32), np.asarray(bk, np.float32),
        np.asarray(Wv, np.float32), np.asarray(bv, np.float32),
        np.asarray(Wo, np.float32), np.asarray(bo, np.float32),
    )
    return out


# revision 9
# speedup vs baseline: 2.5191x; 2.5191x over previous
"""Multi-head attention (B=2, S=2048, D=1024, H=16) on 8 NeuronCores.

Sharding: core c -> batch b = c//4, head group g = c%4 (4 heads each).
Each core computes q/k/v projections for its head group (bias folded in via an
augmented ones-row on x and a bias-row on W), full softmax attention for its 4
heads, and a partial output projection out_c = attn_out_c @ Wo[rows_c].  The
host sums the 4 partials per batch and adds bo.

Kernel layout (per core, all fp32):
  - xT [1152, 2048] = [x.T; ones; zeros]   (K padded 1024+1 -> 9*128)
  - qT/kT [256, 2048] computed directly in transposed layout (heads on
    partitions, 2 tiles of 128 = 2 head-pairs), v in natural layout with an
    extra ones column per head (v_aug) so the attention-output matmul also
    accumulates the softmax denominator as row 64.
  - scoresT[sk, sq] = k @ qT per 128-row k-block, row-tiled 2 heads per pass
    (K=64 each at row groups 0/64); exp on ScalarE straight out of PSUM
    (no max subtraction: scores ~ N(0,1), max << 88).
  - outT[65, sq] accumulates over k-blocks in PSUM; row 64 is the denominator.
    Normalization: reciprocal of row 64, partition-broadcast via DMA, fused
    into the PSUM->SBUF evacuation multiply.
  - projection: out[s, n] = sum_h outTs_h[dv, s].T @ Wo_h[dv, n], K=64 chunks.
"""

import numpy as np

S = 2048
D = 1024
H = 16
DEPTH = 64
NCORES = 8
GH = 4              # heads per core
GD = GH * DEPTH     # 256 output dims per core
KC = 9              # contraction chunks of 128 (1024 data + 1 bias + pad)
KAUG = KC * 128     # 1152

_state = {}


def _build():
    import concourse.mybir as mybir
    import concourse.tile as tile
    from concourse import bacc
    from concourse.bass import ts

    fp32 = mybir.dt.float32
    # All matmul operands live as float32r (same 4-byte layout, np.float32 on
    # the host): the PE streams fp32r at 1 row/cycle vs fp32's 4, at ~tf32
    # operand precision.  PSUM accumulation stays fp32.
    fp32r = mybir.dt.float32r
    Exp = mybir.ActivationFunctionType.Exp

    nc = bacc.Bacc("TRN2", target_bir_lowering=False, debug=False)
    xT = nc.dram_tensor("xT", [KAUG, S], fp32r, kind="ExternalInput")
    wq = nc.dram_tensor("wq", [KAUG, GD], fp32r, kind="ExternalInput")
    wk = nc.dram_tensor("wk", [KAUG, GD], fp32r, kind="ExternalInput")
    wv = nc.dram_tensor("wv", [KAUG, GD], fp32r, kind="ExternalInput")
    wo = nc.dram_tensor("wo", [GD, D], fp32r, kind="ExternalInput")
    out = nc.dram_tensor("out", [S, D], fp32, kind="ExternalOutput")

    with tile.TileContext(nc) as tc:
        with tc.tile_pool(name="singles", bufs=1) as singles:
            qT = singles.tile([128, 2, S], fp32r)       # [dout%128, pair, sq]
            kT = singles.tile([128, 2, S], fp32r)
            v_sb = singles.tile([128, 16, GH, DEPTH + 1], fp32r)  # v_aug
            # rows 0-63: unnormalized attn out^T (normalized in place later);
            # row 64: reciprocal softmax denominators
            outTs = singles.tile([65, GH, S], fp32r)
            nc.vector.memset(v_sb[:, :, :, DEPTH : DEPTH + 1].bitcast(fp32), 1.0)

            # ---------- phase 1: QKV projections ----------
            with (
                tc.tile_pool(name="wpool", bufs=1) as wpool,
                tc.tile_pool(name="xpool", bufs=2) as xpool,
                tc.tile_pool(name="ps1", bufs=3, space="PSUM") as ps1,
            ):
                wq_sb = wpool.tile([128, KC, GD], fp32r)
                wk_sb = wpool.tile([128, KC, GD], fp32r)
                wv_sb = wpool.tile([128, KC, GD], fp32r)
                nc.sync.dma_start(wq_sb[:], wq[:].rearrange("(c p) d -> p c d", p=128))
                nc.sync.dma_start(wk_sb[:], wk[:].rearrange("(c p) d -> p c d", p=128))
                nc.sync.dma_start(wv_sb[:], wv[:].rearrange("(c p) d -> p c d", p=128))
                xT_view = xT[:].rearrange("(c p) s -> p c s", p=128)

                for sc in range(4):  # s-chunks of 512
                    xc = xpool.tile([128, KC, 512], fp32r, tag="xc")
                    nc.sync.dma_start(xc[:], xT_view[:, :, ts(sc, 512)])
                    for hp in range(2):
                        for w_sb, dst in ((wq_sb, qT), (wk_sb, kT)):
                            ps = ps1.tile([128, 512], fp32, tag="pq")
                            for kc in range(KC):
                                nc.tensor.matmul(
                                    ps[:],
                                    w_sb[:, kc, ts(hp, 128)],
                                    xc[:, kc, :],
                                    start=(kc == 0),
                                    stop=(kc == KC - 1),
                                )
                            nc.vector.tensor_copy(dst[:, hp, ts(sc, 512)], ps[:])
                    for mm in range(4):  # s-blocks of 128 inside the chunk
                        ps = ps1.tile([128, GD], fp32, tag="pv")
                        for kc in range(KC):
                            nc.tensor.matmul(
                                ps[:],
                                xc[:, kc, ts(mm, 128)],
                                wv_sb[:, kc, :],
                                start=(kc == 0),
                                stop=(kc == KC - 1),
                            )
                        nc.vector.tensor_copy(
                            v_sb[:, sc * 4 + mm, :, 0:DEPTH],
                            ps[:].rearrange("p (h d) -> p h d", h=GH),
                        )

            # ---------- phase 2: attention ----------
            recip_dram = nc.dram_tensor("recip_scratch", [GH * S], fp32r)
            with (
                tc.tile_pool(name="expp", bufs=3) as expp,
                tc.tile_pool(name="rbp", bufs=1) as rbp,
                tc.tile_pool(name="pss", bufs=2, space="PSUM") as pss,
                tc.tile_pool(name="pso", bufs=4, space="PSUM") as pso,
            ):
                for hp in range(2):
                    for sqc in range(4):
                        oAB = [
                            pso.tile([65, 512], fp32, tag="o", name=f"o{a}")
                            for a in range(2)
                        ]
                        for kb in range(16):
                            sps = pss.tile([128, 2, 512], fp32, tag="s")
                            # scoresT = k @ qT, two heads row-tiled (K=64)
                            for a in range(2):
                                nc.tensor.matmul(
                                    sps[:, a, :],
                                    kT[a * 64 : (a + 1) * 64, hp, ts(kb, 128)],
                                    qT[a * 64 : (a + 1) * 64, hp, ts(sqc, 512)],
                                    start=True,
                                    stop=True,
                                )
                            ex = expp.tile([128, 2, 512], fp32r, tag="e")
                            nc.scalar.activation(ex[:], sps[:], Exp, scale=0.125)
                            # outT[65, sq] += v_aug.T @ expT  (row 64 = denom)
                            for a in range(2):
                                nc.tensor.matmul(
                                    oAB[a][:],
                                    v_sb[:, kb, 2 * hp + a, :],
                                    ex[:, a, :],
                                    start=(kb == 0),
                                    stop=(kb == 15),
                                )
                        for a in range(2):
                            h = 2 * hp + a
                            with nc.allow_low_precision(reason="softmax recip"):
                                nc.vector.reciprocal(
                                    outTs[64:65, h, ts(sqc, 512)], oAB[a][64:65, :]
                                )
                            nc.vector.tensor_copy(
                                outTs[0:64, h, ts(sqc, 512)], oAB[a][0:64, :]
                            )

                # broadcast reciprocal denominators across the 64 dv partitions
                # (SBUF partition-broadcast needs a DRAM bounce)
                rbAll = rbp.tile([64, GH, S], fp32r)
                recip_view = recip_dram[:].rearrange("(p h s) -> p h s", p=1, h=GH)
                nc.sync.dma_start(recip_view, outTs[64:65, :, :])
                nc.sync.dma_start(rbAll[:], recip_view.to_broadcast([64, GH, S]))
                # normalize in place, per s-block so projection can pipeline
                for m in range(16):
                    nc.vector.tensor_mul(
                        outTs[0:64, :, ts(m, 128)],
                        outTs[0:64, :, ts(m, 128)],
                        rbAll[:, :, ts(m, 128)],
                    )

            # ---------- phase 3: output projection ----------
            with (
                tc.tile_pool(name="wop", bufs=1) as wop,
                tc.tile_pool(name="outp", bufs=3) as outp,
                tc.tile_pool(name="ps3", bufs=4, space="PSUM") as ps3,
            ):
                wo_sb = wop.tile([64, GH, D], fp32r)
                nc.sync.dma_start(wo_sb[:], wo[:].rearrange("(h p) n -> p h n", p=64))
                for m in range(16):  # s-blocks of 128
                    ot = outp.tile([128, D], fp32, tag="ot")
                    for nn in range(2):  # n-chunks of 512
                        ps = ps3.tile([128, 512], fp32, tag="po")
                        for h in range(GH):
                            nc.tensor.matmul(
                                ps[:],
                                outTs[0:64, h, ts(m, 128)],
                                wo_sb[:, h, ts(nn, 512)],
                                start=(h == 0),
                                stop=(h == GH - 1),
                            )
                        nc.vector.tensor_copy(ot[:, ts(nn, 512)], ps[:])
                    nc.sync.dma_start(out[m * 128 : (m + 1) * 128, :], ot[:])

    nc.compile()
    return nc


def _get_nc():
    if "nc" not in _state:
        _state["nc"] = _build()
    return _state["nc"]


def _prep_core_inputs(inputs, Wq, bq, Wk, bk, Wv, bv, Wo, bo):
    """Build the 8 per-core input dicts (host-side shard + transpose + bias fold)."""
    in_maps = []
    for c in range(NCORES):
        b, g = divmod(c, 4)
        cols = slice(g * GD, (g + 1) * GD)
        xTa = np.zeros((KAUG, S), np.float32)
        xTa[:D] = inputs[b].T
        xTa[D] = 1.0
        m = {"xT": xTa}
        for name, W, bias in (("wq", Wq, bq), ("wk", Wk, bk), ("wv", Wv, bv)):
            Wa = np.zeros((KAUG, GD), np.float32)
            Wa[:D] = W[:, cols]
            Wa[D] = bias[cols]
            m[name] = Wa
        m["wo"] = np.ascontiguousarray(Wo[cols, :], dtype=np.float32)
        in_maps.append(m)
    return in_maps


def run(inputs, Wq, bq, Wk, bk, Wv, bv, Wo, bo, trace=False):
    from concourse.bass_utils import run_bass_kernel_spmd

    nc = _get_nc()
    in_maps = _prep_core_inputs(inputs, Wq, bq, Wk, bk, Wv, bv, Wo, bo)
    res = run_bass_kernel_spmd(
        nc, in_maps, core_ids=list(range(NCORES)), trace=trace
    )
    out = np.zeros((2, S, D), np.float32)
    for c in range(NCORES):
        out[c // 4] += res.results[c]["out"]
    out += np.asarray(bo, np.float32)
    return out, res


def kernel(inputs, Wq, bq, Wk, bk, Wv, bv, Wo, bo):
    out, _ = run(
        np.asarray(inputs, np.float32),
        np.asarray(Wq, np.float32), np.asarray(bq, np.float32),
        np.asarray(Wk, np.float32), np.asarray(bk, np.float32),
        np.asarray(Wv, np.float32), np.asarray(bv, np.float32),
        np.asarray(Wo, np.float32), np.asarray(bo, np.float32),
    )
    return out
